# revision 59
# baseline (speedup 1.0000x reference)
"""Distributed LlamaAttention (B=2, S=2048, H=2048, 16 heads) on one TRN2 chip.

Sharding: tensor-parallel over heads — core c owns heads (2c, 2c+1).
  * q/k projections: out-feature (head) slices, produced transposed [d, tok]
  * v projection: operand-swapped (hs stationary) to produce natural [tok, d]
  * attention computed with TRANSPOSED scores sT[k, q] (k on partitions), so
    softmax weights come out already in the layout the AV matmul needs —
    no PE transposes at all. Rowsums: DVE accumulates a per-partition
    colsum of the exp tiles (bf16, 2x rate), one [128,1]-ones matmul
    reduces it across partitions, a K=1 matmul broadcasts it back, and a
    128-lane fast reciprocal + multiply normalizes. The whole chain is
    software-pipelined one group behind the PE.
  * o-projection: row-parallel (in-feature slices of wo) -> per-core partials
  * unshard: host sums the 8 partial outputs

All matmuls run in bf16 (TensorE 1 cycle/row) with f32 PSUM accumulation.
Schedule: jq-major attention for both batches with o-projection queued one
jq behind its producing groups, so the dependent tail is only the last jq's
four token tiles. Head DMAs are split across the Sync and Scalar HWDGE
queues so the first matmul isn't gated on a single serialized DMA stream.

Self-contained: hardcodes all shapes; no sibling imports.
"""

import math
from collections import deque

import numpy as np
import ml_dtypes

B, S, HIDDEN, NH, HD = 2, 2048, 2048, 16, 128
N_CORES = 8
HPC = NH // N_CORES          # heads per core = 2
M = HPC * HD                 # per-core projection width = 256
T = B * S                    # 4096 tokens
P = 128                      # partitions
TCH = 512                    # free-dim chunk
KI = HIDDEN // P             # 16 contraction tiles for projections
QT = S // P                  # 16 token tiles per batch elem
JQ = S // TCH                # 4 query chunks per batch elem
BF16 = ml_dtypes.bfloat16

_nc_cache = {}


def _build_nc():
    import concourse.bacc as bacc
    import concourse.mybir as mybir
    from concourse import tile
    from contextlib import ExitStack

    bf = mybir.dt.bfloat16
    f32 = mybir.dt.float32
    AF = mybir.ActivationFunctionType

    nc = bacc.Bacc("TRN2", target_bir_lowering=False, debug=False)

    # Inputs are host-pre-shuffled so every DMA sees long contiguous
    # per-partition runs (multi-KB descriptors instead of 512B ones —
    # the DMA engines are descriptor-rate-bound, not bandwidth-bound).
    hsT = nc.dram_tensor("hsT", [P, 2 * JQ, KI, TCH], bf, kind="ExternalInput").ap()
    wqT = nc.dram_tensor("wqT", [P, KI, M], bf, kind="ExternalInput").ap()
    wkT = nc.dram_tensor("wkT", [P, KI, M], bf, kind="ExternalInput").ap()
    wvT = nc.dram_tensor("wvT", [P, KI, M], bf, kind="ExternalInput").ap()
    woT = nc.dram_tensor("woT", [M, HIDDEN], bf, kind="ExternalInput").ap()
    msk = nc.dram_tensor("maskT", [P, P], bf, kind="ExternalInput").ap()
    iz = nc.dram_tensor("iz", [P, TCH], bf, kind="ExternalInput").ap()
    onc = nc.dram_tensor("onesc", [P, 1], bf, kind="ExternalInput").ap()
    onr = nc.dram_tensor("onesr", [1, P], bf, kind="ExternalInput").ap()
    out = nc.dram_tensor("out", [T, HIDDEN], bf, kind="ExternalOutput").ap()

    out_r = out.rearrange("(n p) o -> p n o", p=P)      # [128, 32, 2048]

    inv_sqrt_d = 1.0 / math.sqrt(HD)

    with tile.TileContext(nc) as tc, ExitStack() as ctx:
        const = ctx.enter_context(tc.tile_pool(name="const", bufs=1))
        qkv = ctx.enter_context(tc.tile_pool(name="qkv", bufs=1))
        hsp = ctx.enter_context(tc.tile_pool(name="hsp", bufs=3))
        expp = ctx.enter_context(tc.tile_pool(name="expp", bufs=2))
        csp = ctx.enter_context(tc.tile_pool(name="csp", bufs=2))
        rcp = ctx.enter_context(tc.tile_pool(name="rcp", bufs=2))
        rbp = ctx.enter_context(tc.tile_pool(name="rbp", bufs=2))
        opl = ctx.enter_context(tc.tile_pool(name="opl", bufs=4))
        mm = ctx.enter_context(tc.tile_pool(name="mm", bufs=2, space="PSUM"))
        sps = ctx.enter_context(tc.tile_pool(name="sps", bufs=3, space="PSUM"))
        avp = ctx.enter_context(tc.tile_pool(name="avp", bufs=2, space="PSUM"))
        rps = ctx.enter_context(tc.tile_pool(name="rps", bufs=1, space="PSUM"))

        # hs chunks split into 4 sub-tiles of 4 KI-slices each so matmul
        # dependencies stay per-DMA precise; same for wq.
        hs_tiles = {}

        def emit_hs_dma(j, eng):
            subs = []
            for g in range(4):
                t = hsp.tile([P, 4, TCH], bf, tag=f"hs{g}", name=f"hs{j}_{g}")
                eng.dma_start(t[:], hsT[:, j, 4 * g:4 * g + 4, :])
                subs.append(t)
            hs_tiles[j] = subs

        # PE warmup: a few matmuls on a zeroed scratch tile so the HAM clock
        # gate flips to 8/8 before the first real (DMA-gated) matmul lands.
        wrm = const.tile([P, P + TCH], bf, name="wrm")
        nc.gpsimd.memset(wrm[:], 0.0)
        wps = mm.tile([P, TCH], f32, tag="mm", name="warm")
        for w in range(28):
            nc.tensor.matmul(
                wps[:], wrm[:, 0:P], wrm[:, P:P + TCH], start=(w == 0), stop=(w == 27)
            )

        # --- head feed: wq/wk/wv on the sync queue, hs chunk 0 on the
        # scalar queue — the two HWDGE queues add DMA throughput while the
        # DMA path is still ramping.
        wq_g = [const.tile([P, 4, M], bf, name=f"wq{g}") for g in range(4)]
        hs0 = []
        for g in range(4):
            nc.sync.dma_start(wq_g[g][:], wqT[:, 4 * g:4 * g + 4, :])
            t = hsp.tile([P, 4, TCH], bf, tag=f"hs{g}", name=f"hs0_{g}")
            nc.scalar.dma_start(t[:], hsT[:, 0, 4 * g:4 * g + 4, :])
            hs0.append(t)
        hs_tiles[0] = hs0
        wk_sb = const.tile([P, KI, M], bf, name="wk_sb")
        wv_sb = const.tile([P, KI, M], bf, name="wv_sb")
        nc.sync.dma_start(wk_sb[:, 0:8, :], wkT[:, 0:8, :])
        nc.sync.dma_start(wk_sb[:, 8:16, :], wkT[:, 8:16, :])
        nc.sync.dma_start(wv_sb[:, 0:8, :], wvT[:, 0:8, :])
        nc.sync.dma_start(wv_sb[:, 8:16, :], wvT[:, 8:16, :])

        # constants needed only from the first attention group onward go on
        # the scalar HWDGE queue behind the hs0 feed.
        wo_sb = const.tile([P, HPC, HIDDEN], bf, name="wo_sb")
        msk_sb = const.tile([P, P], bf, name="msk_sb")
        iz_sb = const.tile([P, TCH], bf, name="iz_sb")
        onc_sb = const.tile([P, 1], bf, name="onc_sb")
        onr_sb = const.tile([1, P], bf, name="onr_sb")

        def emit_const_dmas():
            woT_r = woT.rearrange("(mt p) o -> p mt o", p=P)
            nc.scalar.dma_start(wo_sb[:, 0, :], woT_r[:, 0, :])
            nc.scalar.dma_start(wo_sb[:, 1, :], woT_r[:, 1, :])
            nc.scalar.dma_start(msk_sb[:], msk)
            nc.scalar.dma_start(iz_sb[:], iz)
            nc.scalar.dma_start(onc_sb[:], onc)
            nc.scalar.dma_start(onr_sb[:], onr)

        # --- persistent activations ---
        qT_b = [qkv.tile([P, HPC, S], bf, tag=f"qT{b}", name=f"qT{b}") for b in range(B)]
        kT_b = [qkv.tile([P, HPC, S], bf, tag=f"kT{b}", name=f"kT{b}") for b in range(B)]
        vn_b = [qkv.tile([P, QT, M], bf, tag=f"vn{b}", name=f"vn{b}") for b in range(B)]
        cxT_b = [qkv.tile([P, HPC, S], bf, tag=f"cxT{b}", name=f"cxT{b}") for b in range(B)]

        # --- two-priority filler queues: qkv (hard deps downstream) first ---
        filler_hi = deque()
        filler_lo = deque()
        credit = {"c": 0}
        pace = {"n": 2}                # per-tile filler pop rate

        def pop_filler(n=1):
            credit["c"] += n
            while True:
                q = filler_hi if filler_hi else filler_lo
                if not q or credit["c"] < q[0][0]:
                    break
                c, fn = q.popleft()
                credit["c"] -= c
                fn()

        # ---- QKV projection emission units (6 per 512-token chunk) ----
        def qk_block(b, j4, w_tiles, dst, mt, copy_eng):
            hs_t = hs_tiles[b * JQ + j4]
            ps = mm.tile([P, TCH], f32, tag="mm", name=f"mmqk{b}{j4}{mt}")
            for i in range(KI):
                if isinstance(w_tiles, list):
                    w_ap = w_tiles[i // 4][:, i % 4, mt * P:(mt + 1) * P]
                else:
                    w_ap = w_tiles[:, i, mt * P:(mt + 1) * P]
                nc.tensor.matmul(
                    ps[:],
                    w_ap,
                    hs_t[i // 4][:, i % 4, :],
                    start=(i == 0),
                    stop=(i == KI - 1),
                )
            if copy_eng == "s":
                nc.scalar.copy(dst[:, mt, j4 * TCH:(j4 + 1) * TCH], ps[:])
            else:
                nc.vector.tensor_copy(dst[:, mt, j4 * TCH:(j4 + 1) * TCH], ps[:])

        def v_block(b, j4, tsub, copy_eng):
            hs_t = hs_tiles[b * JQ + j4]
            ps = mm.tile([P, M], f32, tag="mm", name=f"mmv{b}{j4}{tsub}")
            for i in range(KI):
                nc.tensor.matmul(
                    ps[:],
                    hs_t[i // 4][:, i % 4, tsub * P:(tsub + 1) * P],
                    wv_sb[:, i, :],
                    start=(i == 0),
                    stop=(i == KI - 1),
                )
            if copy_eng == "s":
                nc.scalar.copy(vn_b[b][:, j4 * 4 + tsub, :], ps[:])
            else:
                nc.vector.tensor_copy(vn_b[b][:, j4 * 4 + tsub, :], ps[:])

        def qkv_units(b, ce_qk="v", ce_v="v"):
            units = []
            for j4 in range(JQ):
                j = b * JQ + j4
                def u0(b=b, j4=j4, j=j):
                    if j + 1 < 2 * JQ:
                        emit_hs_dma(j + 1, nc.sync)
                    qk_block(b, j4, wq_g, qT_b[b], 0, ce_qk)
                def u1(b=b, j4=j4):
                    qk_block(b, j4, wq_g, qT_b[b], 1, ce_qk)
                def u2(b=b, j4=j4):
                    qk_block(b, j4, wk_sb, kT_b[b], 0, ce_qk)
                def u3(b=b, j4=j4):
                    qk_block(b, j4, wk_sb, kT_b[b], 1, ce_qk)
                def u4(b=b, j4=j4):
                    v_block(b, j4, 0, ce_v)
                    v_block(b, j4, 1, ce_v)
                def u5(b=b, j4=j4):
                    v_block(b, j4, 2, ce_v)
                    v_block(b, j4, 3, ce_v)
                units += [u0, u1, u2, u3, u4, u5]
            return units

        # ---- o-projection micro-units (per 512-wide output chunk) ----
        orow_state = {}

        def oproj_oc(b, tt, oc, split_dma=False, alt_pool=False):
            if oc == 0:
                orow_state[(b, tt)] = opl.tile(
                    [P, HIDDEN], bf, tag="orow", name=f"orow{b}{tt}"
                )
            orow = orow_state[(b, tt)]
            # tail units borrow the (dead) score-psum ring for extra depth
            if alt_pool and oc % 2 == 1:
                ps = sps.tile([P, TCH], f32, tag="s", name=f"mmo{b}{tt}{oc}")
            else:
                ps = mm.tile([P, TCH], f32, tag="mm", name=f"mmo{b}{tt}{oc}")
            for mt in range(HPC):
                nc.tensor.matmul(
                    ps[:],
                    cxT_b[b][:, mt, tt * P:(tt + 1) * P],
                    wo_sb[:, mt, oc * TCH:(oc + 1) * TCH],
                    start=(mt == 0),
                    stop=(mt == HPC - 1),
                )
            if oc % 2 == 0:
                nc.scalar.copy(orow[:, oc * TCH:(oc + 1) * TCH], ps[:])
            else:
                nc.vector.tensor_copy(orow[:, oc * TCH:(oc + 1) * TCH], ps[:])
            if split_dma:
                # final tile: dispatch each 512-col slab as soon as it's copied
                nc.sync.dma_start(
                    out_r[:, b * QT + tt, oc * TCH:(oc + 1) * TCH],
                    orow[:, oc * TCH:(oc + 1) * TCH],
                )
            elif oc == HIDDEN // TCH - 1:
                # one DMA per token tile: 4KB contiguous run per partition
                nc.sync.dma_start(out_r[:, b * QT + tt, :], orow[:])

        def queue_oproj(b, tts, split_dma=False, alt_pool=False):
            for tt in tts:
                for oc in range(HIDDEN // TCH):
                    filler_lo.append(
                        (2, lambda b=b, tt=tt, oc=oc: oproj_oc(
                            b, tt, oc, split_dma, alt_pool))
                    )

        # ---- attention group: transposed-scores flash block ----
        def attn_group_gen(b, h, jq):
            ktmax = 4 * jq + 4
            kts = list(range(4 * jq, 4 * jq + 4)) + list(range(0, 4 * jq))
            expT = expp.tile([P, QT, TCH], bf, tag="exp", name=f"expT{b}{h}{jq}")
            av_ps = avp.tile([P, TCH], f32, tag="av", name=f"av{b}{h}{jq}")
            colsum = csp.tile([P, TCH], bf, tag="cs", name=f"cs{b}{h}{jq}")
            colsum2 = csp.tile([P, TCH], bf, tag="cs2", name=f"cs2{b}{h}{jq}")

            def flush(idx, kt, off):
                nc.tensor.matmul(
                    av_ps[:, off:TCH],
                    vn_b[b][:, kt, h * P:(h + 1) * P],
                    expT[:, kt, off:TCH],
                    start=(idx == 0),
                    stop=(idx == ktmax - 1),
                )

            adds = deque()

            def emit_add():
                # two independent accumulator chains (DVE + GpSimd), merged at
                # end; GpSimd takes 2/3 of the adds (DVE is the busier engine)
                idx, kt, off = adds.popleft()
                if idx == 0:
                    nc.vector.tensor_copy(colsum[:], expT[:, kt, :])
                elif idx == 1:
                    nc.gpsimd.memset(colsum2[:, 0:off], 0.0)
                    nc.gpsimd.tensor_copy(colsum2[:, off:TCH], expT[:, kt, off:TCH])
                elif idx % 3 == 0:
                    nc.vector.tensor_add(
                        colsum[:, off:TCH], colsum[:, off:TCH], expT[:, kt, off:TCH]
                    )
                else:
                    nc.gpsimd.tensor_add(
                        colsum2[:, off:TCH], colsum2[:, off:TCH], expT[:, kt, off:TCH]
                    )

            pend = []

            def tile_post(idx, kt, off):
                pend.append((idx, kt, off))
                adds.append((idx, kt, off))
                if len(pend) > 2:
                    flush(*pend.pop(0))
                if idx >= 2:
                    pop_filler(pace["n"])
                if idx >= 4:
                    emit_add()

            # diagonal tiles: causal mask as a PE accumulate — maskT.T @ [I|0]
            # lays -1e9 on the (q < k) triangle of the diagonal block and
            # exact zeros elsewhere, keeping DVE out of the exp chain.
            for idx in range(4):
                kt = kts[idx]
                off = (kt - 4 * jq) * P
                s_ps = sps.tile([P, TCH], f32, tag="s", name=f"s{b}{h}{jq}{kt}")
                nc.tensor.matmul(
                    s_ps[:, off:TCH],
                    msk_sb[:],
                    iz_sb[:, 0:TCH - off],
                    start=True,
                    stop=False,
                )
                nc.tensor.matmul(
                    s_ps[:, off:TCH],
                    kT_b[b][:, h, kt * P:(kt + 1) * P],
                    qT_b[b][:, h, jq * TCH + off:(jq + 1) * TCH],
                    start=False,
                    stop=True,
                )
                nc.scalar.activation(
                    expT[:, kt, off:TCH], s_ps[:, off:TCH], AF.Exp,
                    scale=inv_sqrt_d,
                )
                tile_post(idx, kt, off)
                if idx == 3:
                    yield None
            for idx in range(4, ktmax):
                kt = kts[idx]
                s_ps = sps.tile([P, TCH], f32, tag="s", name=f"s{b}{h}{jq}{kt}")
                nc.tensor.matmul(
                    s_ps[:],
                    kT_b[b][:, h, kt * P:(kt + 1) * P],
                    qT_b[b][:, h, jq * TCH:(jq + 1) * TCH],
                    start=True,
                    stop=True,
                )
                nc.scalar.activation(
                    expT[:, kt, :], s_ps[:], AF.Exp, scale=inv_sqrt_d
                )
                tile_post(idx, kt, 0)
                if idx == 5:
                    yield None
            for args in pend:
                flush(*args)
            while adds:
                emit_add()
            nc.vector.tensor_add(colsum[:], colsum[:], colsum2[:])
            if ktmax <= 5:
                yield None
            yield (colsum, av_ps)

        def emit_fin2a(p):
            # reduce the colsum across partitions: one ones-matmul per group
            b, h, jq = p["b"], p["h"], p["jq"]
            r_ps = rps.tile([1, TCH], f32, tag="r", name=f"r{b}{h}{jq}")
            nc.tensor.matmul(
                r_ps[0:1, :], onc_sb[:, 0:1], p["colsum"][:], start=True, stop=True
            )
            r_bf = rcp.tile([1, TCH], bf, tag="rcb", name=f"rcb{b}{h}{jq}")
            nc.vector.tensor_copy(r_bf[0:1, :], r_ps[0:1, :])
            p["r_bf"] = r_bf

        def emit_fin2b(p):
            # broadcast rowsums across partitions (K=1 matmul), then 128-lane
            # fast reciprocal and the normalization multiply
            b, h, jq = p["b"], p["h"], p["jq"]
            rb_ps = rps.tile([P, TCH], f32, tag="r", name=f"rb{b}{h}{jq}")
            nc.tensor.matmul(
                rb_ps[:], onr_sb[0:1, :], p["r_bf"][0:1, :], start=True, stop=True
            )
            rb_sb = rbp.tile([P, TCH], f32, tag="rbs", name=f"rbs{b}{h}{jq}")
            nc.vector.reciprocal_approx_fast(rb_sb[:], rb_ps[:])
            nc.vector.tensor_mul(
                cxT_b[b][:, h, jq * TCH:(jq + 1) * TCH], p["av_ps"][:], rb_sb[:]
            )

        prev = {"p": None}

        def run_group(b, h, jq):
            gen = attn_group_gen(b, h, jq)
            next(gen)                      # diagonal tiles emitted
            if prev["p"] is not None:
                emit_fin2a(prev["p"])
            next(gen)
            if prev["p"] is not None:
                emit_fin2b(prev["p"])
            colsum, av_ps = next(gen)
            prev["p"] = {"b": b, "h": h, "jq": jq, "colsum": colsum, "av_ps": av_ps}

        # ---- drive ----
        # Phase A: batch-0 qkv directly (copies on DVE — the scalar queue
        # carries the hs DMA feed and is strict FIFO).
        u0s = qkv_units(0, ce_qk="v", ce_v="v")
        for n, u in enumerate(u0s):
            u()
            if n == 5:
                emit_const_dmas()

        # Phase B: batch-0 attention (jq-major), batch-1 qkv as filler,
        # batch-0 o-projection queued one jq behind.
        for u in qkv_units(1, ce_qk="v", ce_v="v"):
            filler_hi.append((3, u))
        pace["n"] = 1
        for jq in range(JQ):
            if jq == 2:
                pace["n"] = 2
            run_group(0, 0, jq)
            if jq >= 1:
                queue_oproj(0, range(4 * (jq - 1), 4 * jq))
            pop_filler(2)
            run_group(0, 1, jq)
            pop_filler(3)

        # Phase C: batch-1 attention (jq-major), o-projection one jq behind.
        # Pop slower through jq0/jq1 so filler backlog survives into the
        # scalar-bound jq2/jq3 stretch.
        pace["n"] = 1
        run_group(1, 0, 0)
        queue_oproj(0, range(12, QT))
        pop_filler(2)
        run_group(1, 1, 0)
        pop_filler(3)
        for jq in range(1, JQ):
            if jq == 2:
                pace["n"] = 2
            run_group(1, 0, jq)
            # hold back tt11: it bridges the final fin chain's latency
            hi_tt = 4 * jq - (1 if jq == 3 else 0)
            queue_oproj(1, range(4 * (jq - 1), hi_tt))
            pop_filler(2)
            run_group(1, 1, jq)
            pop_filler(3)
        queue_oproj(1, [11])
        pop_filler(2)
        emit_fin2a(prev["p"])
        pop_filler(2)
        emit_fin2b(prev["p"])              # final group's normalization
        queue_oproj(1, range(12, QT - 1), alt_pool=True)
        queue_oproj(1, [QT - 1], split_dma=True, alt_pool=True)
        while filler_hi or filler_lo:
            pop_filler(4)

    nc.compile()
    return nc


def get_nc():
    if "nc" not in _nc_cache:
        _nc_cache["nc"] = _build_nc()
    return _nc_cache["nc"]


def _shuf_w(wT):
    # [HIDDEN, M] with row r = i*P + p  ->  [P, KI, M]
    return np.ascontiguousarray(wT.reshape(KI, P, M).transpose(1, 0, 2)).astype(BF16)


def make_in_maps(hidden_states, wq, wk, wv, wo):
    hs = np.asarray(hidden_states, dtype=np.float32).reshape(T, HIDDEN)
    hsT = np.ascontiguousarray(hs.T)                     # [HIDDEN, T]
    # [P, 2*JQ, KI, TCH]: per-partition 16KB-contiguous chunks
    hsT = np.ascontiguousarray(
        hsT.reshape(KI, P, 2 * JQ, TCH).transpose(1, 2, 0, 3)
    ).astype(BF16)
    # sT orientation: element (k, q) invalid (masked) when q < k.
    # maskT[c, k] = -1e9 for c < k so (maskT.T @ [I|0])[k, q'] masks q' < k.
    maskT = np.triu(np.full((P, P), -1e9, dtype=np.float32), 1).astype(BF16)
    izm = np.zeros((P, TCH), dtype=np.float32)
    izm[:, 0:P] = np.eye(P, dtype=np.float32)
    izm = izm.astype(BF16)
    onesc = np.ones((P, 1), dtype=np.float32).astype(BF16)
    onesr = np.ones((1, P), dtype=np.float32).astype(BF16)
    wq = np.asarray(wq, dtype=np.float32)
    wk = np.asarray(wk, dtype=np.float32)
    wv = np.asarray(wv, dtype=np.float32)
    wo = np.asarray(wo, dtype=np.float32)
    in_maps = []
    for c in range(N_CORES):
        sl = slice(c * M, (c + 1) * M)
        in_maps.append({
            "hsT": hsT,
            "wqT": _shuf_w(wq[sl, :].T),
            "wkT": _shuf_w(wk[sl, :].T),
            "wvT": _shuf_w(wv[sl, :].T),
            "woT": np.ascontiguousarray(wo[:, sl].T).astype(BF16),
            "maskT": maskT,
            "iz": izm,
            "onesc": onesc,
            "onesr": onesr,
        })
    return in_maps


def kernel(hidden_states, wq, wk, wv, wo):
    from concourse.bass_utils import run_bass_kernel_spmd

    nc = get_nc()
    in_maps = make_in_maps(hidden_states, wq, wk, wv, wo)
    res = run_bass_kernel_spmd(nc, in_maps, core_ids=list(range(N_CORES)))
    acc = np.zeros((T, HIDDEN), dtype=np.float32)
    for r in res.results:
        acc += np.asarray(r["out"]).astype(np.float32)
    return acc.reshape(B, S, HIDDEN)


# revision 60
# speedup vs baseline: 1.0133x; 1.0133x over previous
"""Distributed LlamaAttention (B=2, S=2048, H=2048, 16 heads) on one TRN2 chip.

Sharding: tensor-parallel over heads — core c owns heads (2c, 2c+1).
  * q/k projections: out-feature (head) slices, produced transposed [d, tok]
  * v projection: operand-swapped (hs stationary) to produce natural [tok, d]
  * attention computed with TRANSPOSED scores sT[k, q] (k on partitions), so
    softmax weights come out already in the layout the AV matmul needs —
    no PE transposes at all. Rowsums: DVE accumulates a per-partition
    colsum of the exp tiles (bf16, 2x rate), one [128,1]-ones matmul
    reduces it across partitions, a K=1 matmul broadcasts it back, and a
    128-lane fast reciprocal + multiply normalizes. The whole chain is
    software-pipelined one group behind the PE.
  * o-projection: row-parallel (in-feature slices of wo) -> per-core partials
  * unshard: host sums the 8 partial outputs

All matmuls run in bf16 (TensorE 1 cycle/row) with f32 PSUM accumulation.
Schedule: jq-major attention for both batches with o-projection queued one
jq behind its producing groups, so the dependent tail is only the last jq's
four token tiles. Head DMAs are split across the Sync and Scalar HWDGE
queues so the first matmul isn't gated on a single serialized DMA stream.

Self-contained: hardcodes all shapes; no sibling imports.
"""

import math
from collections import deque

import numpy as np
import ml_dtypes

B, S, HIDDEN, NH, HD = 2, 2048, 2048, 16, 128
N_CORES = 8
HPC = NH // N_CORES          # heads per core = 2
M = HPC * HD                 # per-core projection width = 256
T = B * S                    # 4096 tokens
P = 128                      # partitions
TCH = 512                    # free-dim chunk
KI = HIDDEN // P             # 16 contraction tiles for projections
QT = S // P                  # 16 token tiles per batch elem
JQ = S // TCH                # 4 query chunks per batch elem
BF16 = ml_dtypes.bfloat16

_nc_cache = {}


def _build_nc():
    import concourse.bacc as bacc
    import concourse.mybir as mybir
    from concourse import tile
    from contextlib import ExitStack

    bf = mybir.dt.bfloat16
    f32 = mybir.dt.float32
    AF = mybir.ActivationFunctionType

    nc = bacc.Bacc("TRN2", target_bir_lowering=False, debug=False)

    # Inputs are host-pre-shuffled so every DMA sees long contiguous
    # per-partition runs (multi-KB descriptors instead of 512B ones —
    # the DMA engines are descriptor-rate-bound, not bandwidth-bound).
    hsT = nc.dram_tensor("hsT", [P, 2 * JQ, KI, TCH], bf, kind="ExternalInput").ap()
    wqT = nc.dram_tensor("wqT", [P, KI, M], bf, kind="ExternalInput").ap()
    wkT = nc.dram_tensor("wkT", [P, KI, M], bf, kind="ExternalInput").ap()
    wvT = nc.dram_tensor("wvT", [P, KI, M], bf, kind="ExternalInput").ap()
    woT = nc.dram_tensor("woT", [M, HIDDEN], bf, kind="ExternalInput").ap()
    msk = nc.dram_tensor("maskT", [P, P], bf, kind="ExternalInput").ap()
    iz = nc.dram_tensor("iz", [P, TCH], bf, kind="ExternalInput").ap()
    onc = nc.dram_tensor("onesc", [P, 1], bf, kind="ExternalInput").ap()
    onr = nc.dram_tensor("onesr", [1, P], bf, kind="ExternalInput").ap()
    out = nc.dram_tensor("out", [T, HIDDEN], bf, kind="ExternalOutput").ap()

    out_r = out.rearrange("(n p) o -> p n o", p=P)      # [128, 32, 2048]

    inv_sqrt_d = 1.0 / math.sqrt(HD)

    with tile.TileContext(nc) as tc, ExitStack() as ctx:
        const = ctx.enter_context(tc.tile_pool(name="const", bufs=1))
        qkv = ctx.enter_context(tc.tile_pool(name="qkv", bufs=1))
        hsp = ctx.enter_context(tc.tile_pool(name="hsp", bufs=3))
        expp = ctx.enter_context(tc.tile_pool(name="expp", bufs=2))
        csp = ctx.enter_context(tc.tile_pool(name="csp", bufs=2))
        rcp = ctx.enter_context(tc.tile_pool(name="rcp", bufs=2))
        rbp = ctx.enter_context(tc.tile_pool(name="rbp", bufs=2))
        opl = ctx.enter_context(tc.tile_pool(name="opl", bufs=4))
        mm = ctx.enter_context(tc.tile_pool(name="mm", bufs=2, space="PSUM"))
        sps = ctx.enter_context(tc.tile_pool(name="sps", bufs=3, space="PSUM"))
        avp = ctx.enter_context(tc.tile_pool(name="avp", bufs=2, space="PSUM"))
        rps = ctx.enter_context(tc.tile_pool(name="rps", bufs=1, space="PSUM"))

        # hs chunks split into 4 sub-tiles of 4 KI-slices each so matmul
        # dependencies stay per-DMA precise; same for wq.
        hs_tiles = {}

        def emit_hs_dma(j, eng):
            subs = []
            for g in range(4):
                t = hsp.tile([P, 4, TCH], bf, tag=f"hs{g}", name=f"hs{j}_{g}")
                eng.dma_start(t[:], hsT[:, j, 4 * g:4 * g + 4, :])
                subs.append(t)
            hs_tiles[j] = subs

        # PE warmup: a few matmuls on a zeroed scratch tile so the HAM clock
        # gate flips to 8/8 before the first real (DMA-gated) matmul lands.
        wrm = const.tile([P, P + TCH], bf, name="wrm")
        nc.gpsimd.memset(wrm[:], 0.0)
        wps = mm.tile([P, TCH], f32, tag="mm", name="warm")
        for w in range(28):
            nc.tensor.matmul(
                wps[:], wrm[:, 0:P], wrm[:, P:P + TCH], start=(w == 0), stop=(w == 27)
            )

        # --- head feed: wq/wk/wv on the sync queue, hs chunk 0 on the
        # scalar queue — the two HWDGE queues add DMA throughput while the
        # DMA path is still ramping.
        wq_g = [const.tile([P, 4, M], bf, name=f"wq{g}") for g in range(4)]
        hs0 = []
        for g in range(4):
            nc.sync.dma_start(wq_g[g][:], wqT[:, 4 * g:4 * g + 4, :])
            t = hsp.tile([P, 4, TCH], bf, tag=f"hs{g}", name=f"hs0_{g}")
            nc.scalar.dma_start(t[:], hsT[:, 0, 4 * g:4 * g + 4, :])
            hs0.append(t)
        hs_tiles[0] = hs0
        wk_sb = const.tile([P, KI, M], bf, name="wk_sb")
        wv_sb = const.tile([P, KI, M], bf, name="wv_sb")
        nc.sync.dma_start(wk_sb[:, 0:8, :], wkT[:, 0:8, :])
        nc.sync.dma_start(wk_sb[:, 8:16, :], wkT[:, 8:16, :])
        nc.sync.dma_start(wv_sb[:, 0:8, :], wvT[:, 0:8, :])
        nc.sync.dma_start(wv_sb[:, 8:16, :], wvT[:, 8:16, :])

        # constants needed only from the first attention group onward go on
        # the scalar HWDGE queue behind the hs0 feed.
        wo_sb = const.tile([P, HPC, HIDDEN], bf, name="wo_sb")
        msk_sb = const.tile([P, P], bf, name="msk_sb")
        iz_sb = const.tile([P, TCH], bf, name="iz_sb")
        onc_sb = const.tile([P, 1], bf, name="onc_sb")
        onr_sb = const.tile([1, P], bf, name="onr_sb")

        def emit_const_dmas():
            woT_r = woT.rearrange("(mt p) o -> p mt o", p=P)
            nc.scalar.dma_start(wo_sb[:, 0, :], woT_r[:, 0, :])
            nc.scalar.dma_start(wo_sb[:, 1, :], woT_r[:, 1, :])
            nc.scalar.dma_start(msk_sb[:], msk)
            nc.scalar.dma_start(iz_sb[:], iz)
            nc.scalar.dma_start(onc_sb[:], onc)
            nc.scalar.dma_start(onr_sb[:], onr)

        # --- persistent activations ---
        qT_b = [qkv.tile([P, HPC, S], bf, tag=f"qT{b}", name=f"qT{b}") for b in range(B)]
        kT_b = [qkv.tile([P, HPC, S], bf, tag=f"kT{b}", name=f"kT{b}") for b in range(B)]
        vn_b = [qkv.tile([P, QT, M], bf, tag=f"vn{b}", name=f"vn{b}") for b in range(B)]
        cxT_b = [qkv.tile([P, HPC, S], bf, tag=f"cxT{b}", name=f"cxT{b}") for b in range(B)]

        # --- two-priority filler queues: qkv (hard deps downstream) first ---
        filler_hi = deque()
        filler_lo = deque()
        credit = {"c": 0}
        pace = {"n": 2}                # per-tile filler pop rate

        def pop_filler(n=1):
            credit["c"] += n
            while True:
                q = filler_hi if filler_hi else filler_lo
                if not q or credit["c"] < q[0][0]:
                    break
                c, fn = q.popleft()
                credit["c"] -= c
                fn()

        # ---- QKV projection emission units (6 per 512-token chunk) ----
        def qk_block(b, j4, w_tiles, dst, mt, copy_eng):
            hs_t = hs_tiles[b * JQ + j4]
            ps = mm.tile([P, TCH], f32, tag="mm", name=f"mmqk{b}{j4}{mt}")
            for i in range(KI):
                if isinstance(w_tiles, list):
                    w_ap = w_tiles[i // 4][:, i % 4, mt * P:(mt + 1) * P]
                else:
                    w_ap = w_tiles[:, i, mt * P:(mt + 1) * P]
                nc.tensor.matmul(
                    ps[:],
                    w_ap,
                    hs_t[i // 4][:, i % 4, :],
                    start=(i == 0),
                    stop=(i == KI - 1),
                )
            if copy_eng == "s":
                nc.scalar.copy(dst[:, mt, j4 * TCH:(j4 + 1) * TCH], ps[:])
            else:
                nc.vector.tensor_copy(dst[:, mt, j4 * TCH:(j4 + 1) * TCH], ps[:])

        def v_block(b, j4, tsub, copy_eng):
            hs_t = hs_tiles[b * JQ + j4]
            ps = mm.tile([P, M], f32, tag="mm", name=f"mmv{b}{j4}{tsub}")
            for i in range(KI):
                nc.tensor.matmul(
                    ps[:],
                    hs_t[i // 4][:, i % 4, tsub * P:(tsub + 1) * P],
                    wv_sb[:, i, :],
                    start=(i == 0),
                    stop=(i == KI - 1),
                )
            if copy_eng == "s":
                nc.scalar.copy(vn_b[b][:, j4 * 4 + tsub, :], ps[:])
            else:
                nc.vector.tensor_copy(vn_b[b][:, j4 * 4 + tsub, :], ps[:])

        def qkv_units(b, ce_qk="v", ce_v="v"):
            units = []
            for j4 in range(JQ):
                j = b * JQ + j4
                def u0(b=b, j4=j4, j=j):
                    if j + 1 < 2 * JQ:
                        emit_hs_dma(j + 1, nc.sync)
                    qk_block(b, j4, wq_g, qT_b[b], 0, ce_qk)
                def u1(b=b, j4=j4):
                    qk_block(b, j4, wq_g, qT_b[b], 1, ce_qk)
                def u2(b=b, j4=j4):
                    qk_block(b, j4, wk_sb, kT_b[b], 0, ce_qk)
                def u3(b=b, j4=j4):
                    qk_block(b, j4, wk_sb, kT_b[b], 1, ce_qk)
                def u4(b=b, j4=j4):
                    v_block(b, j4, 0, ce_v)
                    v_block(b, j4, 1, ce_v)
                def u5(b=b, j4=j4):
                    v_block(b, j4, 2, ce_v)
                    v_block(b, j4, 3, ce_v)
                units += [u0, u1, u2, u3, u4, u5]
            return units

        # ---- o-projection micro-units (per 512-wide output chunk) ----
        orow_state = {}

        def oproj_oc(b, tt, oc, split_dma=False, alt_pool=False):
            if oc == 0:
                orow_state[(b, tt)] = opl.tile(
                    [P, HIDDEN], bf, tag="orow", name=f"orow{b}{tt}"
                )
            orow = orow_state[(b, tt)]
            # tail units borrow the (dead) score-psum ring for extra depth
            if alt_pool and oc % 2 == 1:
                ps = sps.tile([P, TCH], f32, tag="s", name=f"mmo{b}{tt}{oc}")
            else:
                ps = mm.tile([P, TCH], f32, tag="mm", name=f"mmo{b}{tt}{oc}")
            for mt in range(HPC):
                nc.tensor.matmul(
                    ps[:],
                    cxT_b[b][:, mt, tt * P:(tt + 1) * P],
                    wo_sb[:, mt, oc * TCH:(oc + 1) * TCH],
                    start=(mt == 0),
                    stop=(mt == HPC - 1),
                )
            if oc % 2 == 0:
                nc.scalar.copy(orow[:, oc * TCH:(oc + 1) * TCH], ps[:])
            else:
                nc.vector.tensor_copy(orow[:, oc * TCH:(oc + 1) * TCH], ps[:])
            if split_dma:
                # final tile: dispatch each 512-col slab as soon as it's copied
                nc.sync.dma_start(
                    out_r[:, b * QT + tt, oc * TCH:(oc + 1) * TCH],
                    orow[:, oc * TCH:(oc + 1) * TCH],
                )
            elif oc == HIDDEN // TCH - 1:
                # one DMA per token tile: 4KB contiguous run per partition
                nc.sync.dma_start(out_r[:, b * QT + tt, :], orow[:])

        def queue_oproj(b, tts, split_dma=False, alt_pool=False):
            for tt in tts:
                for oc in range(HIDDEN // TCH):
                    filler_lo.append(
                        (2, lambda b=b, tt=tt, oc=oc: oproj_oc(
                            b, tt, oc, split_dma, alt_pool))
                    )

        # ---- attention group: transposed-scores flash block ----
        def attn_group_gen(b, h, jq):
            ktmax = 4 * jq + 4
            kts = list(range(4 * jq, 4 * jq + 4)) + list(range(0, 4 * jq))
            expT = expp.tile([P, QT, TCH], bf, tag="exp", name=f"expT{b}{h}{jq}")
            av_ps = avp.tile([P, TCH], f32, tag="av", name=f"av{b}{h}{jq}")
            colsum = csp.tile([P, TCH], bf, tag="cs", name=f"cs{b}{h}{jq}")
            colsum2 = csp.tile([P, TCH], bf, tag="cs2", name=f"cs2{b}{h}{jq}")

            def flush(idx, kt, off):
                nc.tensor.matmul(
                    av_ps[:, off:TCH],
                    vn_b[b][:, kt, h * P:(h + 1) * P],
                    expT[:, kt, off:TCH],
                    start=(idx == 0),
                    stop=(idx == ktmax - 1),
                )

            adds = deque()

            def emit_add():
                # two independent accumulator chains (DVE + GpSimd), merged at
                # end; GpSimd takes 2/3 of the adds (DVE is the busier engine)
                idx, kt, off = adds.popleft()
                if idx == 0:
                    nc.vector.tensor_copy(colsum[:], expT[:, kt, :])
                elif idx == 1:
                    nc.gpsimd.memset(colsum2[:, 0:off], 0.0)
                    nc.gpsimd.tensor_copy(colsum2[:, off:TCH], expT[:, kt, off:TCH])
                elif idx % 3 == 0:
                    nc.vector.tensor_add(
                        colsum[:, off:TCH], colsum[:, off:TCH], expT[:, kt, off:TCH]
                    )
                else:
                    nc.gpsimd.tensor_add(
                        colsum2[:, off:TCH], colsum2[:, off:TCH], expT[:, kt, off:TCH]
                    )

            pend = []

            def tile_post(idx, kt, off):
                pend.append((idx, kt, off))
                adds.append((idx, kt, off))
                if len(pend) > 2:
                    flush(*pend.pop(0))
                if idx >= 2:
                    pop_filler(pace["n"])
                if idx >= 4:
                    emit_add()

            # diagonal tiles: causal mask as a PE accumulate — maskT.T @ [I|0]
            # lays -1e9 on the (q < k) triangle of the diagonal block and
            # exact zeros elsewhere, keeping DVE out of the exp chain.
            for idx in range(4):
                kt = kts[idx]
                off = (kt - 4 * jq) * P
                s_ps = sps.tile([P, TCH], f32, tag="s", name=f"s{b}{h}{jq}{kt}")
                nc.tensor.matmul(
                    s_ps[:, off:TCH],
                    msk_sb[:],
                    iz_sb[:, 0:TCH - off],
                    start=True,
                    stop=False,
                )
                nc.tensor.matmul(
                    s_ps[:, off:TCH],
                    kT_b[b][:, h, kt * P:(kt + 1) * P],
                    qT_b[b][:, h, jq * TCH + off:(jq + 1) * TCH],
                    start=False,
                    stop=True,
                )
                nc.scalar.activation(
                    expT[:, kt, off:TCH], s_ps[:, off:TCH], AF.Exp,
                    scale=inv_sqrt_d,
                )
                tile_post(idx, kt, off)
                if idx == 3:
                    yield None
            for idx in range(4, ktmax):
                kt = kts[idx]
                s_ps = sps.tile([P, TCH], f32, tag="s", name=f"s{b}{h}{jq}{kt}")
                nc.tensor.matmul(
                    s_ps[:],
                    kT_b[b][:, h, kt * P:(kt + 1) * P],
                    qT_b[b][:, h, jq * TCH:(jq + 1) * TCH],
                    start=True,
                    stop=True,
                )
                nc.scalar.activation(
                    expT[:, kt, :], s_ps[:], AF.Exp, scale=inv_sqrt_d
                )
                tile_post(idx, kt, 0)
                if idx == 5:
                    yield None
            for args in pend:
                flush(*args)
            while adds:
                emit_add()
            nc.vector.tensor_add(colsum[:], colsum[:], colsum2[:])
            if ktmax <= 5:
                yield None
            yield (colsum, av_ps)

        def emit_fin2a(p):
            # reduce the colsum across partitions: one ones-matmul per group
            b, h, jq = p["b"], p["h"], p["jq"]
            r_ps = rps.tile([1, TCH], f32, tag="r", name=f"r{b}{h}{jq}")
            nc.tensor.matmul(
                r_ps[0:1, :], onc_sb[:, 0:1], p["colsum"][:], start=True, stop=True
            )
            r_bf = rcp.tile([1, TCH], bf, tag="rcb", name=f"rcb{b}{h}{jq}")
            nc.vector.tensor_copy(r_bf[0:1, :], r_ps[0:1, :])
            p["r_bf"] = r_bf

        def emit_fin2b(p):
            # broadcast rowsums across partitions (K=1 matmul), then 128-lane
            # fast reciprocal and the normalization multiply
            b, h, jq = p["b"], p["h"], p["jq"]
            rb_ps = rps.tile([P, TCH], f32, tag="r", name=f"rb{b}{h}{jq}")
            nc.tensor.matmul(
                rb_ps[:], onr_sb[0:1, :], p["r_bf"][0:1, :], start=True, stop=True
            )
            rb_sb = rbp.tile([P, TCH], f32, tag="rbs", name=f"rbs{b}{h}{jq}")
            nc.vector.reciprocal_approx_fast(rb_sb[:], rb_ps[:])
            nc.vector.tensor_mul(
                cxT_b[b][:, h, jq * TCH:(jq + 1) * TCH], p["av_ps"][:], rb_sb[:]
            )

        prev = {"p": None}

        def run_group(b, h, jq):
            gen = attn_group_gen(b, h, jq)
            next(gen)                      # diagonal tiles emitted
            if prev["p"] is not None:
                emit_fin2a(prev["p"])
            next(gen)
            if prev["p"] is not None:
                emit_fin2b(prev["p"])
            colsum, av_ps = next(gen)
            prev["p"] = {"b": b, "h": h, "jq": jq, "colsum": colsum, "av_ps": av_ps}

        # ---- drive ----
        # Phase A: batch-0 qkv directly (copies on DVE — the scalar queue
        # carries the hs DMA feed and is strict FIFO).
        u0s = qkv_units(0, ce_qk="v", ce_v="v")
        for n, u in enumerate(u0s):
            u()
            if n == 5:
                emit_const_dmas()

        # Phase B: batch-0 attention (jq-major), batch-1 qkv as filler,
        # batch-0 o-projection queued one jq behind.
        for u in qkv_units(1, ce_qk="v", ce_v="v"):
            filler_hi.append((3, u))
        for jq in range(JQ):
            run_group(0, 0, jq)
            if jq >= 1:
                queue_oproj(0, range(4 * (jq - 1), 4 * jq))
            pop_filler(2)
            run_group(0, 1, jq)
            pop_filler(3)

        # Phase C: batch-1 attention (jq-major), o-projection one jq behind.
        # Pop slower through jq0/jq1 so filler backlog survives into the
        # scalar-bound jq2/jq3 stretch.
        pace["n"] = 1
        run_group(1, 0, 0)
        queue_oproj(0, range(12, QT))
        pop_filler(2)
        run_group(1, 1, 0)
        pop_filler(3)
        for jq in range(1, JQ):
            if jq == 2:
                pace["n"] = 2
            run_group(1, 0, jq)
            # hold back tt11: it bridges the final fin chain's latency
            hi_tt = 4 * jq - (1 if jq == 3 else 0)
            queue_oproj(1, range(4 * (jq - 1), hi_tt))
            pop_filler(2)
            run_group(1, 1, jq)
            pop_filler(3)
        queue_oproj(1, [11])
        pop_filler(2)
        emit_fin2a(prev["p"])
        pop_filler(2)
        emit_fin2b(prev["p"])              # final group's normalization
        queue_oproj(1, range(12, QT - 1), alt_pool=True)
        queue_oproj(1, [QT - 1], split_dma=True, alt_pool=True)
        while filler_hi or filler_lo:
            pop_filler(4)

    nc.compile()
    return nc


def get_nc():
    if "nc" not in _nc_cache:
        _nc_cache["nc"] = _build_nc()
    return _nc_cache["nc"]


def _shuf_w(wT):
    # [HIDDEN, M] with row r = i*P + p  ->  [P, KI, M]
    return np.ascontiguousarray(wT.reshape(KI, P, M).transpose(1, 0, 2)).astype(BF16)


def make_in_maps(hidden_states, wq, wk, wv, wo):
    hs = np.asarray(hidden_states, dtype=np.float32).reshape(T, HIDDEN)
    hsT = np.ascontiguousarray(hs.T)                     # [HIDDEN, T]
    # [P, 2*JQ, KI, TCH]: per-partition 16KB-contiguous chunks
    hsT = np.ascontiguousarray(
        hsT.reshape(KI, P, 2 * JQ, TCH).transpose(1, 2, 0, 3)
    ).astype(BF16)
    # sT orientation: element (k, q) invalid (masked) when q < k.
    # maskT[c, k] = -1e9 for c < k so (maskT.T @ [I|0])[k, q'] masks q' < k.
    maskT = np.triu(np.full((P, P), -1e9, dtype=np.float32), 1).astype(BF16)
    izm = np.zeros((P, TCH), dtype=np.float32)
    izm[:, 0:P] = np.eye(P, dtype=np.float32)
    izm = izm.astype(BF16)
    onesc = np.ones((P, 1), dtype=np.float32).astype(BF16)
    onesr = np.ones((1, P), dtype=np.float32).astype(BF16)
    wq = np.asarray(wq, dtype=np.float32)
    wk = np.asarray(wk, dtype=np.float32)
    wv = np.asarray(wv, dtype=np.float32)
    wo = np.asarray(wo, dtype=np.float32)
    in_maps = []
    for c in range(N_CORES):
        sl = slice(c * M, (c + 1) * M)
        in_maps.append({
            "hsT": hsT,
            "wqT": _shuf_w(wq[sl, :].T),
            "wkT": _shuf_w(wk[sl, :].T),
            "wvT": _shuf_w(wv[sl, :].T),
            "woT": np.ascontiguousarray(wo[:, sl].T).astype(BF16),
            "maskT": maskT,
            "iz": izm,
            "onesc": onesc,
            "onesr": onesr,
        })
    return in_maps


def kernel(hidden_states, wq, wk, wv, wo):
    from concourse.bass_utils import run_bass_kernel_spmd

    nc = get_nc()
    in_maps = make_in_maps(hidden_states, wq, wk, wv, wo)
    res = run_bass_kernel_spmd(nc, in_maps, core_ids=list(range(N_CORES)))
    acc = np.zeros((T, HIDDEN), dtype=np.float32)
    for r in res.results:
        acc += np.asarray(r["out"]).astype(np.float32)
    return acc.reshape(B, S, HIDDEN)
